# revision 2
# baseline (speedup 1.0000x reference)
"""Trainium2 Bass kernel v2 for CrossCondGPT2 block-sparse attention.

Same contract/host-side sharding as kernel.py (4 batches x 2 head-groups,
6 heads per core). Differences from v1:
  - ST phase is k-stationary with wide-N run pieces; the two heads of a
    chunk occupy partition halves 0-63 / 64-127 so their K=64 matmuls
    row-tile into the PE array and stream concurrently.
  - exp evacuation is one wide ACTIVATE per bin over both heads' banks.
  - PV is k-major, accumulating into 4 persistent PSUM banks per head
    (bank = 512-row block of q; the c=0 pieces cover each bank full-width
    so the scattered accumulation starts clean).
  - outputs are written as two bf16 partials (head-chunks 0+1 early,
    chunk 2 in the tail); the host sums them in f32.
  - PE/ACT warm-up during the input DMA; inputs arrive over 4 queues in
    first-needed order.
"""

import sys

sys.path.insert(0, "/opt/trn_rl_repo")

import numpy as np
import ml_dtypes

import concourse.bass as bass
import concourse.tile as tile
from concourse import bacc, mybir
from concourse.bass_utils import run_bass_kernel_spmd

BF16 = mybir.dt.bfloat16
F32 = mybir.dt.float32
NPBF = ml_dtypes.bfloat16

L, C, T, HD = 1616, 768, 512, 64
NH, NHL = 12, 6
DL = NHL * HD
LT = 13
CK = C // 128
MK = DL // 128
LCHUNKS = [(0, 512), (512, 512), (1024, 512), (1536, 80)]
NEG = -1.0e9
EXP_FUNC = mybir.ActivationFunctionType.Exp
ADD_OP = None  # set lazily from mybir


def _bins():
    """Static schedule: per k-tile c, contiguous q-runs (ST layout), split
    into <=512 pieces and packed into per-c bins (one PSUM bank per head).
    mask 'c'/'s' applies to the first 128 columns of the marked piece."""
    runs = {c: [] for c in range(13)}
    for c in range(4):
        w = (4 - c) * 128
        runs[c] = [(c * 128, w, 'c'), (512 + c * 128, w, 'c'),
                   (1024 + c * 128, w, 'c')]
    for c in range(4, 8):
        t = c - 4
        w = (4 - t) * 128
        runs[c] = [(512 + t * 128, w, 's'), (1024 + t * 128, w, 'c')]
    for c in range(8, 12):
        t = c - 8
        w = (4 - t) * 128
        runs[c] = [(512 + t * 128, w, 's'), (1024 + t * 128, w, 's')]
    runs[12] = [(512, 1104, None)]

    bins = []
    for c in range(13):
        kw = 128 if c < 12 else 80
        pieces = []
        for (q, w, mk) in runs[c]:
            while w > 512:
                pieces.append((q, 512, mk))
                q += 512
                w -= 512
                mk = None
            pieces.append((q, w, mk))
        cur, curw = [], 0
        for (q, w, mk) in pieces:
            if curw + w > 512:
                bins.append(dict(c=c, kw=kw, pieces=cur))
                cur, curw = [], 0
            cur.append((curw, q, w, mk))
            curw += w
        if cur:
            bins.append(dict(c=c, kw=kw, pieces=cur))

    # first/last flags per y-bank (PV accumulation groups, c-major order)
    bank_seq = {b: [] for b in range(4)}
    for bi, b in enumerate(bins):
        for (off, q, w, mk) in b['pieces']:
            bank_seq[q // 512].append((bi, off))
    first = {lst[0] for lst in bank_seq.values() if lst}
    last = {lst[-1] for lst in bank_seq.values() if lst}
    for bi, b in enumerate(bins):
        pl = []
        for (off, q, w, mk) in b['pieces']:
            pl.append(dict(off=off, q=q, w=w, mk=mk, bank=q // 512,
                           boff=q % 512, first=(bi, off) in first,
                           last=(bi, off) in last))
        b['pieces'] = pl
    return bins


BINS = _bins()


def build_nc():
    nc = bacc.Bacc("TRN2", target_bir_lowering=False, debug=False,
                   num_devices=8)

    xT_d = nc.dram_tensor("xT", [C, L], BF16, kind="ExternalInput").ap()
    wq_d = nc.dram_tensor("wqT", [C, DL], BF16, kind="ExternalInput").ap()
    wk_d = nc.dram_tensor("wkT", [C, DL], BF16, kind="ExternalInput").ap()
    wv_d = nc.dram_tensor("wvT", [C, DL], BF16, kind="ExternalInput").ap()
    wp_d = nc.dram_tensor("wpT", [DL, C], BF16, kind="ExternalInput").ap()
    bq_d = nc.dram_tensor("bqs", [DL], F32, kind="ExternalInput").ap()
    bk_d = nc.dram_tensor("bks", [DL], F32, kind="ExternalInput").ap()
    uc_d = nc.dram_tensor("u_c", [128, 128], BF16, kind="ExternalInput").ap()
    us_d = nc.dram_tensor("u_s", [128, 128], BF16, kind="ExternalInput").ap()
    vt_d = nc.dram_tensor("v_tri", [128, 128], BF16, kind="ExternalInput").ap()
    outa_d = nc.dram_tensor("out_a", [L, C], BF16, kind="ExternalOutput").ap()
    outb_d = nc.dram_tensor("out_b", [L, C], BF16, kind="ExternalOutput").ap()

    with tile.TileContext(nc) as tc:
        with (
            tc.tile_pool(name="persist", bufs=1) as persist,
            tc.tile_pool(name="sbw", bufs=2) as sbw,
            tc.tile_pool(name="expp", bufs=26) as expp,
            tc.tile_pool(name="outs", bufs=3) as outs,
            tc.tile_pool(name="dramp", bufs=2, space="DRAM") as dramp,
            tc.tile_pool(name="psst", bufs=2, space="PSUM") as ps_st,
            tc.tile_pool(name="psy", bufs=1, space="PSUM") as ps_y,
        ):
            wq_sb = persist.tile([128, CK, DL], BF16)
            wk_sb = persist.tile([128, CK, DL], BF16)
            wv_sb = persist.tile([128, CK, DL], BF16)
            wp_sb = persist.tile([128, MK, C], BF16)
            bq_sb = persist.tile([128, MK], F32)
            bk_sb = persist.tile([128, MK], F32)
            uc_sb = persist.tile([128, 128], BF16)
            us_sb = persist.tile([128, 128], BF16)
            vt_sb = persist.tile([128, 128], BF16)
            xT = persist.tile([128, CK, L], BF16)
            qt = persist.tile([128, MK, L], BF16)
            kt = persist.tile([128, MK, L], BF16)
            vsb = persist.tile([128, LT, NHL, HD + 1], BF16)
            yt_all = persist.tile([128, MK, L], BF16)
            ones_sb = persist.tile([1, 64], F32)
            warm_sb = persist.tile([128, 512], BF16)
            wscr = persist.tile([128, 8], F32)

            # ---- warm-up: HAM clock gate + exp table load, while inputs DMA
            nc.vector.memset(ones_sb[0:1, 0:64], 1.0)
            nc.vector.memset(warm_sb[0:128, 0:512], 0.0)
            wps = ps_st.tile([128, 2, 512], F32, tag="st")
            for i in range(8):
                nc.tensor.matmul(wps[0:128, i % 2, 0:512],
                                 warm_sb[0:128, 0:128], warm_sb[0:128, 0:512],
                                 start=True, stop=True)
            nc.scalar.activation(wscr[0:128, 0:8], wps[0:128, 0, 0:8],
                                 EXP_FUNC)

            # ---- input DMAs: 4 queues, first-needed-first
            xT_r = xT_d.rearrange("(k p) n -> p k n", p=128)
            wq_r = wq_d.rearrange("(k p) n -> p k n", p=128)
            wk_r = wk_d.rearrange("(k p) n -> p k n", p=128)
            wv_r = wv_d.rearrange("(k p) n -> p k n", p=128)
            nc.gpsimd.dma_start(bq_sb[:], bq_d.rearrange("(m p) -> p m", p=128))
            nc.gpsimd.dma_start(bk_sb[:], bk_d.rearrange("(m p) -> p m", p=128))
            nc.sync.dma_start(wq_sb[:], wq_r[:, :, :])
            nc.sync.dma_start(xT[:, :, 0:512], xT_r[:, :, 0:512])
            nc.scalar.dma_start(wk_sb[:], wk_r[:, :, :])
            nc.scalar.dma_start(xT[:, :, 512:1024], xT_r[:, :, 512:1024])
            nc.sync.dma_start(xT[:, :, 1024:1616], xT_r[:, :, 1024:1616])
            nc.scalar.dma_start(wv_sb[:], wv_r[:, :, :])
            nc.gpsimd.dma_start(uc_sb[:], uc_d[:])
            nc.gpsimd.dma_start(us_sb[:], us_d[:])
            nc.gpsimd.dma_start(vt_sb[:], vt_d[:])
            nc.gpsimd.dma_start(wp_sb[:], wp_d.rearrange("(k p) n -> p k n", p=128))

            def proj_qk_unit(proj, m, lo, lwc):
                wsb, bsb, dst, scale = (
                    (wq_sb, bq_sb, qt, 0.125) if proj == 0
                    else (wk_sb, bk_sb, kt, 1.0))
                pmq = ps_st.tile([128, 2, 512], F32, tag="st", name="pmq")
                for kk in range(CK):
                    nc.tensor.matmul(
                        pmq[0:128, 0, 0:lwc],
                        wsb[:, kk, m * 128:(m + 1) * 128],
                        xT[:, kk, lo:lo + lwc],
                        start=(kk == 0), stop=(kk == CK - 1))
                nc.vector.tensor_scalar(
                    dst[:, m, lo:lo + lwc], pmq[0:128, 0, 0:lwc],
                    bsb[:, m:m + 1], scale,
                    mybir.AluOpType.add, mybir.AluOpType.mult)

            def proj_v_unit(lt):
                lw = 128 if lt < 12 else 80
                pmv = ps_st.tile([128, 2, 512], F32, tag="st", name="pmv")
                for kk in range(CK):
                    nc.tensor.matmul(
                        pmv[0:lw, 0, 0:DL],
                        xT[:, kk, lt * 128:lt * 128 + lw],
                        wv_sb[:, kk, :],
                        start=(kk == 0), stop=(kk == CK - 1))
                nc.vector.tensor_copy(
                    vsb[0:lw, lt, :, 0:HD],
                    pmv[0:lw, 0, 0:DL].rearrange("p (h d) -> p h d", h=NHL))
                nc.vector.memset(vsb[0:lw, lt, :, HD:HD + 1], 1.0)

            def st_bin(m, b):
                c, kw = b['c'], b['kw']
                stp = ps_st.tile([128, 2, 512], F32, tag="st", name="stp")
                for p in b['pieces']:
                    for hslot in (0, 1):
                        hs = hslot * 64
                        nc.tensor.matmul(
                            stp[0:kw, hslot, p['off']:p['off'] + p['w']],
                            kt[hs:hs + 64, m, c * 128:c * 128 + kw],
                            qt[hs:hs + 64, m, p['q']:p['q'] + p['w']],
                            start=True, stop=True)
                    if p['mk'] is not None:
                        for hslot in (0, 1):
                            nc.tensor.matmul(
                                stp[0:128, hslot, p['off']:p['off'] + 128],
                                uc_sb[:] if p['mk'] == 'c' else us_sb[:],
                                vt_sb[0:128, 0:128],
                                start=False, stop=True,
                                skip_group_check=True)
                w_tot = p['off'] + p['w']
                ex = expp.tile([128, 2, 512], BF16, tag="exps", name="ex")
                nc.scalar.activation(ex[0:kw, :, 0:w_tot],
                                     stp[0:kw, :, 0:w_tot], EXP_FUNC)
                return ex

            def pv_bin(m, hslot, b, y, ex):
                c, kw = b['c'], b['kw']
                lh = 2 * m + hslot
                for p in b['pieces']:
                    nc.tensor.matmul(
                        y[0:65, p['bank'], p['boff']:p['boff'] + p['w']],
                        vsb[0:kw, c, lh, 0:65],
                        ex[0:kw, hslot, p['off']:p['off'] + p['w']],
                        start=p['first'], stop=p['last'],
                        skip_group_check=True)

            def evac_norm(m, hslot, y, pe_path):
                ys = sbw.tile([65, L], F32, tag="ystage", name="ys")
                for bnk, (qs, wq_) in enumerate(
                        [(0, 512), (512, 512), (1024, 512), (1536, 80)]):
                    nc.vector.tensor_copy(ys[0:65, qs:qs + wq_],
                                          y[0:65, bnk, 0:wq_])
                srec = sbw.tile([101, 16], F32, tag="srec", name="srec")
                nc.scalar.dma_start(srec[0:101, :], ys[64:65, :])
                nc.vector.reciprocal(srec[0:101, :], srec[0:101, :])
                hs = hslot * 64
                if not pe_path:
                    rec_d = dramp.tile([1, L], F32, tag="recd", name="recd")
                    nc.gpsimd.dma_start(rec_d[0:1, :], srec[0:101, :])
                    recb = sbw.tile([64, L], F32, tag="recb", name="recb")
                    nc.gpsimd.dma_start(recb[0:64, :],
                                        rec_d[0:1, :].to_broadcast((64, L)))
                    nc.gpsimd.tensor_mul(yt_all[hs:hs + 64, m, :],
                                         ys[0:64, :], recb[0:64, :])
                else:
                    rr = sbw.tile([1, L], F32, tag="recrow", name="rr")
                    nc.sync.dma_start(rr[0:1, :], srec[0:101, :])
                    for lo, lwc in LCHUNKS:
                        bc = ps_st.tile([128, 2, 512], F32, tag="st",
                                        name="bc")
                        nc.tensor.matmul(bc[0:64, 0, 0:lwc],
                                         ones_sb[0:1, 0:64],
                                         rr[0:1, lo:lo + lwc],
                                         start=True, stop=True)
                        nc.vector.tensor_mul(
                            yt_all[hs:hs + 64, m, lo:lo + lwc],
                            ys[0:64, lo:lo + lwc], bc[0:64, 0, 0:lwc])

            def outproj_unit(r, part):
                qw = 128 if r < 12 else 80
                qsl = slice(r * 128, r * 128 + qw)
                ost = outs.tile([128, C], BF16, tag=f"o{part}", name="ost")
                for no, nw in ((0, 512), (512, 256)):
                    pmo = ps_st.tile([128, 2, 512], F32, tag="st", name="pmo")
                    if part == 0:
                        for kk in (0, 1):
                            nc.tensor.matmul(
                                pmo[0:qw, 0, 0:nw], yt_all[:, kk, qsl],
                                wp_sb[:, kk, no:no + nw],
                                start=(kk == 0), stop=(kk == 1))
                    else:
                        nc.tensor.matmul(
                            pmo[0:qw, 0, 0:nw], yt_all[:, 2, qsl],
                            wp_sb[:, 2, no:no + nw], start=True, stop=True)
                    nc.vector.tensor_copy(ost[0:qw, no:no + nw],
                                          pmo[0:qw, 0, 0:nw])
                eng = nc.sync if r % 2 == 0 else nc.scalar
                eng.dma_start((outa_d if part == 0 else outb_d)[qsl, :],
                              ost[0:qw, :])

            LAG = 2

            def pair_attn(m, fillers, start_at=0):
                nfb = len(BINS)
                fN = len(fillers)
                fi = 0
                exs = []
                y0 = ps_y.tile([65, 4, 512], F32, tag="y", name="y0")
                done = 0
                for j, b in enumerate(BINS):
                    exs.append(st_bin(m, b))
                    if j >= start_at and nfb > start_at:
                        want = (j + 1 - start_at) * fN // (nfb - start_at)
                        while fi < min(want, fN):
                            fillers[fi]()
                            fi += 1
                    while done <= j - LAG:
                        pv_bin(m, 0, BINS[done], y0, exs[done])
                        done += 1
                while done < nfb:
                    pv_bin(m, 0, BINS[done], y0, exs[done])
                    done += 1
                evac_norm(m, 0, y0, pe_path=False)
                while fi < fN:
                    fillers[fi]()
                    fi += 1
                y1 = ps_y.tile([65, 4, 512], F32, tag="y", name="y1")
                for j, b in enumerate(BINS):
                    pv_bin(m, 1, b, y1, exs[j])
                evac_norm(m, 1, y1, pe_path=(m == 2))

            # ---- main sequence ----
            for pj in (0, 1):
                for lo, lwc in LCHUNKS:
                    proj_qk_unit(pj, 0, lo, lwc)

            f0 = ([(lambda lt=lt: proj_v_unit(lt)) for lt in range(LT)] +
                  [(lambda a=a, lo=lo, lwc=lwc: proj_qk_unit(a, 1, lo, lwc))
                   for a in (0, 1) for (lo, lwc) in LCHUNKS])
            pair_attn(0, f0)

            f1 = [(lambda a=a, lo=lo, lwc=lwc: proj_qk_unit(a, 2, lo, lwc))
                  for a in (0, 1) for (lo, lwc) in LCHUNKS]
            pair_attn(1, f1)

            f2 = [(lambda r=r: outproj_unit(r, 0)) for r in range(LT)]
            pair_attn(2, f2, start_at=10)

            for r in range(LT):
                outproj_unit(r, 1)

    nc.compile()
    return nc


_NC_CACHE = None


def _get_nc():
    global _NC_CACHE
    if _NC_CACHE is None:
        _NC_CACHE = build_nc()
    return _NC_CACHE


def make_in_maps(inputs):
    x = np.asarray(inputs["x"], np.float32)
    Wq = np.asarray(inputs["Wq"], np.float32)
    Wk = np.asarray(inputs["Wk"], np.float32)
    Wv = np.asarray(inputs["Wv"], np.float32)
    Wp = np.asarray(inputs["Wp"], np.float32)
    bq = np.asarray(inputs["bq"], np.float32)
    bk = np.asarray(inputs["bk"], np.float32)

    u_c = (NEG * np.eye(128, k=1)).astype(NPBF)
    u_s = (NEG * np.eye(128)).astype(NPBF)
    v_tri = np.tril(np.ones((128, 128))).astype(NPBF)

    in_maps = []
    for b in range(4):
        xT_b = np.ascontiguousarray(x[b].T).astype(NPBF)
        for g in range(2):
            sl = slice(g * DL, (g + 1) * DL)
            in_maps.append({
                "xT": xT_b,
                "wqT": np.ascontiguousarray(Wq[sl, :].T).astype(NPBF),
                "wkT": np.ascontiguousarray(Wk[sl, :].T).astype(NPBF),
                "wvT": np.ascontiguousarray(Wv[sl, :].T).astype(NPBF),
                "wpT": np.ascontiguousarray(Wp[:, sl].T).astype(NPBF),
                "bqs": np.ascontiguousarray(bq[sl]).astype(np.float32),
                "bks": np.ascontiguousarray(bk[sl]).astype(np.float32),
                "u_c": u_c,
                "u_s": u_s,
                "v_tri": v_tri,
            })
    return in_maps


def combine_outputs(inputs, results):
    Wp = np.asarray(inputs["Wp"], np.float32)
    bv = np.asarray(inputs["bv"], np.float32)
    bp = np.asarray(inputs["bp"], np.float32)
    bp_eff = bp + Wp @ bv
    parts = [np.asarray(r["out_a"], np.float32) +
             np.asarray(r["out_b"], np.float32) for r in results]
    out = np.stack(
        [parts[2 * b] + parts[2 * b + 1] + bp_eff for b in range(4)]
    ).astype(np.float32)
    return out


def run(inputs, trace=False):
    nc = _get_nc()
    in_maps = make_in_maps(inputs)
    res = run_bass_kernel_spmd(
        nc, in_maps, core_ids=list(range(8)), trace=trace
    )
    return combine_outputs(inputs, res.results), res


def kernel(**inputs):
    out, _ = run(inputs, trace=False)
    return out
